# revision 3
# baseline (speedup 1.0000x reference)
"""Dense transformer block (B=4, T=2048, C=1024, H=16, FF=4096) on 8
Trainium2 NeuronCores — v2.

Sharding: sequence-parallel, zero collectives. Core c handles batch
b = c // 2 and query-token half r = c % 2. The host permutes each
core's token order so its OWN 1024 query tokens are always columns
[0:1024) (zigzag: r=0 owns global [0:512)+[1536:2048), r=1 owns
[512:1536)). K/V/LN1 are computed redundantly for the full permuted
sequence, entirely in SBUF (no DRAM bounces).

All matmul operands are bf16 (fp32 PSUM accumulate); LN statistics in
fp32. LN affine params are folded into the projection weights host-side
(the K-side bias is dropped entirely: a per-query constant logit shift
cancels in softmax). Causality:
  - 8 triangular chunk-slots use bf16 masks multiplied post-exp (DVE)
  - core-dependent all-zero/all-one chunk-slots use pre-flagged V
    copies (vt2/vt3), so no mask work at all
K-projection is interleaved per-head-pair with attention so the PE
stays busy during the exp-bound softmax stream.
"""
import numpy as np
import ml_dtypes

B, T, C = 4, 2048, 1024
H, D, FF = 16, 64, 4096
NC = 8
NKC = C // 128     # 8 feature chunks
NFFC = FF // 128   # 32
OWN = 1024
EPS = 1e-5

_STATE = {}


def _build_program():
    import concourse.bacc as bacc
    import concourse.mybir as mybir
    from concourse.tile import TileContext

    F32R = mybir.dt.float32r
    F32 = mybir.dt.float32
    BF16 = mybir.dt.bfloat16
    AF = mybir.ActivationFunctionType
    OP = mybir.AluOpType

    nc = bacc.Bacc("TRN2", target_bir_lowering=False, debug=False,
                   num_devices=NC)

    xbf_d = nc.dram_tensor("xbf", [128, NKC, T], BF16, kind="ExternalInput")
    xq_d = nc.dram_tensor("xq", [128, NKC, OWN], F32, kind="ExternalInput")
    wq_d = nc.dram_tensor("wq", [8, 128, NKC, 128], BF16, kind="ExternalInput")
    wk_d = nc.dram_tensor("wk", [8, 128, NKC, 128], BF16, kind="ExternalInput")
    wv_d = nc.dram_tensor("wv", [128, NKC, C], BF16, kind="ExternalInput")
    wp_d = nc.dram_tensor("wp", [8, 128, NKC, 128], BF16, kind="ExternalInput")
    wf1_d = nc.dram_tensor("wf1", [NFFC, 128, NKC, 128], BF16,
                           kind="ExternalInput")
    wf2_d = nc.dram_tensor("wf2", [NKC, 128, NFFC, 128], BF16,
                           kind="ExternalInput")
    qb_d = nc.dram_tensor("qb", [128, 8], F32, kind="ExternalInput")
    bp_d = nc.dram_tensor("bp", [128, NKC], F32, kind="ExternalInput")
    bf1_d = nc.dram_tensor("bf1", [128, NFFC], F32, kind="ExternalInput")
    bf2_d = nc.dram_tensor("bf2", [128, NKC], F32, kind="ExternalInput")
    # 4 triangular mask slots (same pattern set on every core)
    mask_d = nc.dram_tensor("masks", [128, 4, 512], BF16,
                            kind="ExternalInput")
    fa_d = nc.dram_tensor("fa", [128, 64], F32, kind="ExternalInput")
    fb_d = nc.dram_tensor("fb", [128, 64], F32, kind="ExternalInput")
    out_d = nc.dram_tensor("out", [128, NKC, OWN], F32, kind="ExternalOutput")

    def mm(ps, lhsT, rhs, start, stop):
        nc.tensor.matmul(ps, lhsT, rhs, start=start, stop=stop)

    with TileContext(nc, pool_alloc_mode="queue") as tc:
        consts_cm = tc.tile_pool(name="consts", bufs=1)
        consts = consts_cm.__enter__()

        ones128 = consts.tile([128, 1], F32R)
        nc.vector.memset(ones128.bitcast(F32), 1.0)
        ones128b = consts.tile([128, 1], BF16)
        nc.vector.memset(ones128b, 1.0)
        onesrow = consts.tile([1, 128], F32R)
        nc.vector.memset(onesrow.bitcast(F32), 1.0)
        negrow = consts.tile([1, 128], F32R)
        nc.vector.memset(negrow.bitcast(F32), -1.0)
        eps_t = consts.tile([1, 1], F32)
        nc.vector.memset(eps_t, EPS)
        qb_t = consts.tile([128, 8], F32)
        nc.sync.dma_start(out=qb_t, in_=qb_d[:, :])
        bp_t = consts.tile([128, NKC], F32)
        nc.sync.dma_start(out=bp_t, in_=bp_d[:, :])
        bf1_t = consts.tile([128, NFFC], F32)
        nc.sync.dma_start(out=bf1_t, in_=bf1_d[:, :])
        bf2_t = consts.tile([128, NKC], F32)
        nc.sync.dma_start(out=bf2_t, in_=bf2_d[:, :])
        fa_t = consts.tile([128, 64], F32)
        nc.sync.dma_start(out=fa_t, in_=fa_d[:, :])
        fb_t = consts.tile([128, 64], F32)
        nc.sync.dma_start(out=fb_t, in_=fb_d[:, :])

        # x2t/yt outlive the attention pools -> their pools open first
        x2_cm = tc.tile_pool(name="x2p", bufs=1)
        x2p = x2_cm.__enter__()
        x2t = x2p.tile([128, NKC, OWN], F32R)
        yt_cm = tc.tile_pool(name="ytp", bufs=1)
        ytp = yt_cm.__enter__()
        yt = ytp.tile([128, NKC, OWN], BF16)

        # attention-lifetime activation tiles (closed after proj)
        pers_cm = tc.tile_pool(name="pers", bufs=1)
        pers = pers_cm.__enter__()
        lnx = [pers.tile([128, NKC, 512], BF16, tag=f"lnx{tb}",
                         name=f"lnx{tb}") for tb in range(4)]
        vt = [pers.tile([128, 16, 8, 65], BF16, tag=f"vt{g}",
                        name=f"vt{g}") for g in range(2)]
        vt2 = pers.tile([128, 4, 16, 65], BF16, tag="vt2")
        vt3 = pers.tile([128, 4, 16, 65], BF16, tag="vt3")
        qts = [pers.tile([128, OWN], BF16, tag=f"q{hp}", name=f"q{hp}")
               for hp in range(8)]

        # ------- Phases A-C interleaved: LN1 + V + Q per token-block -------
        xtb_cm = tc.tile_pool(name="xtbp", bufs=2)
        xtbp = xtb_cm.__enter__()
        lw_cm = tc.tile_pool(name="lnw", bufs=2)
        lnw = lw_cm.__enter__()
        lst_cm = tc.tile_pool(name="lnst", bufs=1)
        lst = lst_cm.__enter__()
        wvp_cm = tc.tile_pool(name="wvp", bufs=1)
        wvp = wvp_cm.__enter__()
        wv_t = wvp.tile([128, NKC, C], BF16)
        nc.sync.dma_start(out=wv_t, in_=wv_d[:, :, :])
        qw_cm = tc.tile_pool(name="qw", bufs=3)
        qw = qw_cm.__enter__()
        lps_cm = tc.tile_pool(name="lnps", bufs=1, space="PSUM")
        lps = lps_cm.__enter__()
        pps_cm = tc.tile_pool(name="projps", bufs=2, space="PSUM")
        pjps = pps_cm.__enter__()

        nc.vector.memset(vt[0][:, :, :, 64:65], 1.0)
        nc.vector.memset(vt[1][:, :, :, 64:65], 1.0)
        nc.gpsimd.tensor_copy(
            out=vt2[:, :, :, 64:65].rearrange("p a h o -> p (a h o)"), in_=fa_t)
        nc.gpsimd.tensor_copy(
            out=vt3[:, :, :, 64:65].rearrange("p a h o -> p (a h o)"), in_=fb_t)

        def ln1_tb(tb):
            sl = slice(tb * 512, (tb + 1) * 512)
            xtb = xtbp.tile([128, NKC, 512], BF16, tag="xtb")
            nc.sync.dma_start(out=xtb, in_=xbf_d[:, :, sl])
            ps_s = lps.tile([1, 512], F32, tag="ps_s")
            for k in range(NKC):
                mm(ps_s, ones128b, xtb[:, k, :], k == 0, k == NKC - 1)
            sqs = []
            for k in range(NKC):
                sq = lnw.tile([128, 512], BF16, tag=f"sq{k % 2}")
                nc.scalar.activation(out=sq, in_=xtb[:, k, :], func=AF.Square)
                sqs.append(sq)
            ps_q = lps.tile([1, 512], F32, tag="ps_q")
            for k in range(NKC):
                mm(ps_q, ones128b, sqs[k], k == 0, k == NKC - 1)
            mu = lst.tile([1, 512], F32R, tag="mu")
            nc.vector.tensor_scalar_mul(out=mu, in0=ps_s, scalar1=1.0 / C)
            msq = lst.tile([1, 512], F32, tag="msq")
            nc.vector.tensor_scalar_mul(out=msq, in0=ps_q, scalar1=1.0 / C)
            var = lst.tile([1, 512], F32, tag="var")
            nc.vector.tensor_mul(out=var, in0=mu.bitcast(F32),
                                 in1=mu.bitcast(F32))
            nc.vector.tensor_sub(out=var, in0=msq, in1=var)
            nc.scalar.activation(out=msq, in_=var, func=AF.Sqrt,
                                 bias=eps_t, scale=1.0)
            rstd = lst.tile([1, 512], F32R, tag="rstd")
            with nc.allow_low_precision(reason="f32r rstd"):
                nc.vector.reciprocal(out=rstd, in_=msq)
            pmr = lst.tile([1, 512], F32R, tag="pmr")
            with nc.allow_low_precision(reason="f32r pmr"):
                nc.vector.tensor_mul(out=pmr, in0=mu.bitcast(F32),
                                     in1=rstd.bitcast(F32))
            ps_r = lps.tile([128, 512], F32, tag="ps_r")
            mm(ps_r, onesrow, rstd, True, True)
            ps_n = lps.tile([128, 512], F32, tag="ps_n")
            mm(ps_n, negrow, pmr, True, True)
            rb_s = lnw.tile([128, 512], BF16, tag="rb_s")
            nc.scalar.activation(out=rb_s, in_=ps_r, func=AF.Copy)
            nb_s = lnw.tile([128, 512], BF16, tag="nb_s")
            nc.scalar.activation(out=nb_s, in_=ps_n, func=AF.Copy)
            for k in range(NKC):
                nc.vector.tensor_mul(out=lnx[tb][:, k, :], in0=xtb[:, k, :],
                                     in1=rb_s)
                nc.vector.tensor_add(out=lnx[tb][:, k, :],
                                     in0=lnx[tb][:, k, :], in1=nb_s)

        def v_chunk(cch):
            tbi, coff = cch // 4, (cch % 4) * 128
            ps_v = pjps.tile([128, C], F32, tag="pv")
            for h2 in range(2):
                vsl = slice(h2 * 512, (h2 + 1) * 512)
                for k in range(NKC):
                    mm(ps_v[:, vsl], lnx[tbi][:, k, coff:coff + 128],
                       wv_t[:, k, vsl], k == 0, k == NKC - 1)
            for g in range(2):
                nc.vector.tensor_copy(
                    out=vt[g][:, cch, :, 0:64],
                    in_=ps_v[:, g * 512:(g + 1) * 512]
                    .rearrange("p (h d) -> p h d", h=8))
            if 8 <= cch < 12:
                nc.vector.tensor_scalar_mul(
                    out=vt2[:, cch - 8, :, 0:64],
                    in0=ps_v.rearrange("p (h d) -> p h d", h=16),
                    scalar1=fa_t[:, 0:1])
            if 12 <= cch:
                nc.vector.tensor_scalar_mul(
                    out=vt3[:, cch - 12, :, 0:64],
                    in0=ps_v.rearrange("p (h d) -> p h d", h=16),
                    scalar1=fb_t[:, 0:1])

        def q_hp(hp):
            wt = qw.tile([128, NKC, 128], BF16, tag="w")
            nc.sync.dma_start(out=wt, in_=wq_d[hp])
            ps = pjps.tile([128, OWN], F32, tag="pv")
            for tb in range(2):
                sl = slice(tb * 512, (tb + 1) * 512)
                for k in range(NKC):
                    mm(ps[:, sl], wt[:, k, :], lnx[tb][:, k, :],
                       k == 0, k == NKC - 1)
            nc.vector.tensor_scalar_add(out=qts[hp], in0=ps,
                                        scalar1=qb_t[:, hp:hp + 1])

        for tb in range(4):
            ln1_tb(tb)
            if tb >= 1:
                for cch in range(4 * (tb - 1), 4 * tb):
                    v_chunk(cch)
            if tb == 2:
                for hp in range(4):
                    q_hp(hp)
            if tb == 3:
                for hp in range(4, 8):
                    q_hp(hp)
        for cch in range(12, 16):
            v_chunk(cch)

        pps_cm.__exit__(None, None, None)
        lps_cm.__exit__(None, None, None)
        qw_cm.__exit__(None, None, None)
        wvp_cm.__exit__(None, None, None)
        lst_cm.__exit__(None, None, None)
        lw_cm.__exit__(None, None, None)
        xtb_cm.__exit__(None, None, None)

        # ---------- Phase D: K projection + attention, per head-pair ----------
        # qb0 slots: chunks 0..3 (tri, mask slot=ci), 8..11 (flagged vt2)
        # qb1 slots: chunks 0..3, 8..11 (plain), 4..7 (tri, slot=ci-4),
        #            12..15 (flagged vt3)
        SLOTS = {
            0: [(ci, "tri", ci) for ci in range(4)]
               + [(ci, "v2", None) for ci in range(8, 12)],
            1: [(ci, "one", None) for ci in range(4)]
               + [(ci, "tri", ci - 4) for ci in range(4, 8)]
               + [(ci, "one", None) for ci in range(8, 12)]
               + [(ci, "v3", None) for ci in range(12, 16)],
        }

        kw_cm = tc.tile_pool(name="kw", bufs=3)
        kw = kw_cm.__enter__()
        mask_t = kw.tile([128, 4, 512], BF16, tag="mask", bufs=1)
        nc.sync.dma_start(out=mask_t, in_=mask_d[:, :, :])
        kt_cm = tc.tile_pool(name="ktp", bufs=3)
        ktp = kt_cm.__enter__()
        att_cm = tc.tile_pool(name="attw", bufs=3)
        attw = att_cm.__enter__()
        sm_cm = tc.tile_pool(name="smw", bufs=2)
        smw = sm_cm.__enter__()
        kps_cm = tc.tile_pool(name="kps", bufs=2, space="PSUM")
        kps = kps_cm.__enter__()
        sps_cm = tc.tile_pool(name="sps", bufs=2, space="PSUM")
        sps = sps_cm.__enter__()
        yps_cm = tc.tile_pool(name="yps", bufs=1, space="PSUM")
        yps = yps_cm.__enter__()

        for hp in range(8):
            ha, hb = 2 * hp, 2 * hp + 1
            g = hp // 4
            wt = kw.tile([128, NKC, 128], BF16, tag="w")
            nc.sync.dma_start(out=wt, in_=wk_d[hp])
            kt_t = ktp.tile([128, T], BF16, tag="kt")
            for tb in range(4):
                sl = slice(tb * 512, (tb + 1) * 512)
                ps_k = kps.tile([128, 512], F32, tag="mm")
                for k in range(NKC):
                    mm(ps_k, wt[:, k, :], lnx[tb][:, k, :],
                       k == 0, k == NKC - 1)
                nc.vector.tensor_copy(out=kt_t[:, sl], in_=ps_k)

            for qb in range(2):
                slots = SLOTS[qb]
                nsl = len(slots)
                qsl = slice(qb * 512, (qb + 1) * 512)
                ps_y = yps.tile([65, 2, 512], F32, tag="y")
                for idx, (ci, kind, mslot) in enumerate(slots):
                    csl = slice(ci * 128, (ci + 1) * 128)
                    ps_s = sps.tile([128, 2, 512], F32, tag="s")
                    mm(ps_s[:, 0, :], kt_t[0:64, csl], qts[hp][0:64, qsl],
                       True, True)
                    mm(ps_s[:, 1, :], kt_t[64:128, csl], qts[hp][64:128, qsl],
                       True, True)
                    ptm = attw.tile([128, 2, 512], BF16, tag="ptm")
                    if kind == "tri":
                        pt = attw.tile([128, 2, 512], BF16, tag="pt")
                        nc.scalar.activation(out=pt, in_=ps_s, func=AF.Exp)
                        nc.vector.tensor_mul(
                            out=ptm, in0=pt,
                            in1=mask_t[:, mslot:mslot + 1, :]
                            .broadcast_to([128, 2, 512]))
                    else:
                        nc.scalar.activation(out=ptm, in_=ps_s, func=AF.Exp)
                    if kind == "v2":
                        va, vb = vt2[:, ci - 8, ha, :], vt2[:, ci - 8, hb, :]
                    elif kind == "v3":
                        va, vb = vt3[:, ci - 12, ha, :], vt3[:, ci - 12, hb, :]
                    else:
                        va = vt[g][:, ci, ha - g * 8, :]
                        vb = vt[g][:, ci, hb - g * 8, :]
                    mm(ps_y[:, 0, :], va, ptm[:, 0, :], idx == 0,
                       idx == nsl - 1)
                    mm(ps_y[:, 1, :], vb, ptm[:, 1, :], idx == 0,
                       idx == nsl - 1)
                for hh, h in ((0, ha), (1, hb)):
                    rd = smw.tile([1, 512], F32R, tag="rd")
                    with nc.allow_low_precision(reason="softmax denom"):
                        nc.vector.reciprocal(out=rd, in_=ps_y[64:65, hh, :])
                    rb = smw.tile([64, 512], F32, tag="rb")
                    nc.gpsimd.partition_broadcast(rb, rd.bitcast(F32))
                    nc.vector.tensor_mul(
                        out=yt[64 * hh:64 * hh + 64, hp, qsl],
                        in0=ps_y[0:64, hh, :], in1=rb)

        yps_cm.__exit__(None, None, None)
        sps_cm.__exit__(None, None, None)
        kps_cm.__exit__(None, None, None)
        sm_cm.__exit__(None, None, None)
        att_cm.__exit__(None, None, None)
        kt_cm.__exit__(None, None, None)
        kw_cm.__exit__(None, None, None)
        pers_cm.__exit__(None, None, None)

        # ------- Phase E+F: proj + residual + LN2, per 512-token half -------
        l2x_cm = tc.tile_pool(name="l2x", bufs=1)
        l2xp = l2x_cm.__enter__()
        ln2x = l2xp.tile([128, NKC, OWN], BF16)
        pw_cm = tc.tile_pool(name="pw", bufs=3)
        pw = pw_cm.__enter__()
        l2w_cm = tc.tile_pool(name="l2w", bufs=2)
        l2w = l2w_cm.__enter__()
        pps_cm = tc.tile_pool(name="pps", bufs=2, space="PSUM")
        pps = pps_cm.__enter__()
        l2ps_cm = tc.tile_pool(name="l2ps", bufs=1, space="PSUM")
        l2ps = l2ps_cm.__enter__()

        for h2 in range(2):
            tsl = slice(h2 * 512, (h2 + 1) * 512)
            for oc in range(8):
                wt = pw.tile([128, NKC, 128], BF16, tag=f"w{h2}")
                nc.sync.dma_start(out=wt, in_=wp_d[oc])
                xqo = pw.tile([128, 512], F32, tag="xq")
                nc.sync.dma_start(out=xqo, in_=xq_d[:, oc, tsl])
                ps = pps.tile([128, 512], F32, tag="mm")
                for k in range(NKC):
                    mm(ps, wt[:, k, :], yt[:, k, tsl], k == 0, k == NKC - 1)
                with nc.allow_low_precision(reason="f32r x2"):
                    nc.vector.scalar_tensor_tensor(
                        out=x2t[:, oc, tsl], in0=ps,
                        scalar=bp_t[:, oc:oc + 1],
                        in1=xqo, op0=OP.add, op1=OP.add)
            # LN2 on this half
            ps_s2 = l2ps.tile([1, 512], F32, tag="ps_s")
            for k in range(NKC):
                mm(ps_s2, ones128, x2t[:, k, tsl], k == 0, k == NKC - 1)
            sq2s = []
            for k in range(NKC):
                sq = l2w.tile([128, 512], BF16, tag=f"sq{k % 2}")
                nc.scalar.activation(out=sq, in_=x2t[:, k, tsl].bitcast(F32),
                                     func=AF.Square)
                sq2s.append(sq)
            ps_q2 = l2ps.tile([1, 512], F32, tag="ps_q")
            for k in range(NKC):
                mm(ps_q2, ones128b, sq2s[k], k == 0, k == NKC - 1)
            mu2_ = l2w.tile([1, 512], F32R, tag="mu")
            nc.vector.tensor_scalar_mul(out=mu2_, in0=ps_s2, scalar1=1.0 / C)
            msq2 = l2w.tile([1, 512], F32, tag="msq")
            nc.vector.tensor_scalar_mul(out=msq2, in0=ps_q2, scalar1=1.0 / C)
            var2 = l2w.tile([1, 512], F32, tag="var")
            nc.vector.tensor_mul(out=var2, in0=mu2_.bitcast(F32),
                                 in1=mu2_.bitcast(F32))
            nc.vector.tensor_sub(out=var2, in0=msq2, in1=var2)
            nc.scalar.activation(out=msq2, in_=var2, func=AF.Sqrt,
                                 bias=eps_t, scale=1.0)
            rstd2 = l2w.tile([1, 512], F32R, tag="rstd")
            with nc.allow_low_precision(reason="f32r rstd"):
                nc.vector.reciprocal(out=rstd2, in_=msq2)
            pmr2 = l2w.tile([1, 512], F32R, tag="pmr")
            with nc.allow_low_precision(reason="f32r pmr"):
                nc.vector.tensor_mul(out=pmr2, in0=mu2_.bitcast(F32),
                                     in1=rstd2.bitcast(F32))
            ps_r2 = l2ps.tile([128, 512], F32, tag="ps_r")
            mm(ps_r2, onesrow, rstd2, True, True)
            ps_n2 = l2ps.tile([128, 512], F32, tag="ps_n")
            mm(ps_n2, negrow, pmr2, True, True)
            nb2 = l2w.tile([128, 512], BF16, tag="nb2")
            nc.scalar.activation(out=nb2, in_=ps_n2, func=AF.Copy)
            for k in range(NKC):
                nc.vector.tensor_mul(out=ln2x[:, k, tsl],
                                     in0=x2t[:, k, tsl].bitcast(F32),
                                     in1=ps_r2)
                nc.vector.tensor_add(out=ln2x[:, k, tsl],
                                     in0=ln2x[:, k, tsl], in1=nb2)

        l2ps_cm.__exit__(None, None, None)
        pps_cm.__exit__(None, None, None)
        l2w_cm.__exit__(None, None, None)
        pw_cm.__exit__(None, None, None)

        # ---------------- Phase G: MLP ----------------
        m1_cm = tc.tile_pool(name="m1p", bufs=1)
        m1p = m1_cm.__enter__()
        m1t = m1p.tile([128, NFFC, OWN], BF16)
        mw_cm = tc.tile_pool(name="mw", bufs=3)
        mw = mw_cm.__enter__()
        mo_cm = tc.tile_pool(name="mo", bufs=2)
        mo = mo_cm.__enter__()
        mps_cm = tc.tile_pool(name="mps", bufs=2, space="PSUM")
        mps = mps_cm.__enter__()

        for ffc in range(NFFC):
            wt = mw.tile([128, NKC, 128], BF16, tag="w1")
            nc.sync.dma_start(out=wt, in_=wf1_d[ffc])
            ps = mps.tile([128, OWN], F32, tag="mm1")
            for h2 in range(2):
                tsl = slice(h2 * 512, (h2 + 1) * 512)
                for k in range(NKC):
                    mm(ps[:, tsl], wt[:, k, :], ln2x[:, k, tsl],
                       k == 0, k == NKC - 1)
            nc.scalar.activation(out=m1t[:, ffc, :], in_=ps,
                                 func=AF.Relu,
                                 bias=bf1_t[:, ffc:ffc + 1], scale=1.0)
        for oc in range(NKC):
            wt2 = mw.tile([128, NFFC, 128], BF16, tag="w2")
            nc.sync.dma_start(out=wt2, in_=wf2_d[oc])
            ps = mps.tile([128, OWN], F32, tag="mm2")
            for h2 in range(2):
                tsl = slice(h2 * 512, (h2 + 1) * 512)
                for k in range(NFFC):
                    mm(ps[:, tsl], wt2[:, k, :], m1t[:, k, tsl],
                       k == 0, k == NFFC - 1)
            ot = mo.tile([128, OWN], F32, tag="ot")
            nc.vector.scalar_tensor_tensor(
                out=ot, in0=ps, scalar=bf2_t[:, oc:oc + 1],
                in1=x2t[:, oc, :].bitcast(F32), op0=OP.add, op1=OP.add)
            nc.sync.dma_start(out=out_d[:, oc, :], in_=ot)

        mps_cm.__exit__(None, None, None)
        mo_cm.__exit__(None, None, None)
        mw_cm.__exit__(None, None, None)
        m1_cm.__exit__(None, None, None)
        l2x_cm.__exit__(None, None, None)
        yt_cm.__exit__(None, None, None)
        x2_cm.__exit__(None, None, None)
        consts_cm.__exit__(None, None, None)

    nc.compile()
    return nc


def _perm(r):
    if r == 0:
        return np.concatenate([np.arange(0, 512), np.arange(1536, 2048),
                               np.arange(512, 1536)])
    return np.concatenate([np.arange(512, 1536), np.arange(0, 512),
                           np.arange(1536, 2048)])


def _prep_in_maps(x, W_attn, W_proj, b_proj, W_fc1, b_fc1, W_fc2, b_fc2,
                  ln1_g, ln1_b, ln2_g, ln2_b):
    f32 = np.float32
    bf16 = ml_dtypes.bfloat16
    x = np.asarray(x, f32)
    W_attn = np.asarray(W_attn, f32)
    Wq, Wk, Wv = W_attn[:, 0:C], W_attn[:, C:2 * C], W_attn[:, 2 * C:3 * C]
    W_proj = np.asarray(W_proj, f32)
    W_fc1 = np.asarray(W_fc1, f32)
    W_fc2 = np.asarray(W_fc2, f32)
    g1 = np.asarray(ln1_g, f32)
    b1 = np.asarray(ln1_b, f32)
    g2 = np.asarray(ln2_g, f32)
    b2 = np.asarray(ln2_b, f32)

    s = 1.0 / np.sqrt(D)
    Wq_f = (g1[:, None] * Wq) * s
    Wk_f = g1[:, None] * Wk
    Wv_f = g1[:, None] * Wv
    Wf1_f = g2[:, None] * W_fc1
    qbias = (b1 @ Wq) * s                      # [C]
    vbias = b1 @ Wv                            # [C]
    bp_f = np.asarray(b_proj, f32) + vbias @ W_proj
    bf1_f = np.asarray(b_fc1, f32) + b2 @ W_fc1
    bf2_f = np.asarray(b_fc2, f32)

    def lhs_tiles(W, nout):
        nin = W.shape[0] // 128
        return np.ascontiguousarray(
            W.reshape(nin, 128, nout, 128).transpose(2, 1, 0, 3)
        ).astype(bf16)

    def vec(v, nk):
        return np.ascontiguousarray(np.asarray(v, f32).reshape(nk, 128).T)

    kvp = np.arange(128)
    qi = np.arange(512)
    masks = np.zeros((128, 4, 512), np.float32)
    for j in range(4):
        masks[:, j, :] = (128 * j + kvp[:, None]) <= qi[None, :]

    shared = {
        "wq": lhs_tiles(Wq_f, 8), "wk": lhs_tiles(Wk_f, 8),
        "wv": np.ascontiguousarray(
            Wv_f.reshape(NKC, 128, C).transpose(1, 0, 2)).astype(bf16),
        "wp": lhs_tiles(W_proj, 8),
        "wf1": lhs_tiles(Wf1_f, NFFC), "wf2": lhs_tiles(W_fc2, NKC),
        "qb": vec(qbias, 8), "bp": vec(bp_f, NKC),
        "bf1": vec(bf1_f, NFFC), "bf2": vec(bf2_f, NKC),
        "masks": masks.astype(bf16),
    }

    in_maps = []
    for c in range(NC):
        b, r = c // 2, c % 2
        perm = _perm(r)
        xs = x[b][perm]                       # [T, C] permuted
        xt = np.ascontiguousarray(
            xs.T.reshape(NKC, 128, T).transpose(1, 0, 2))
        fa = np.full((128, 64), float(r == 1), np.float32)
        fb = np.full((128, 64), float(r == 0), np.float32)
        d = {"xbf": xt.astype(bf16), "xq": np.ascontiguousarray(xt[:, :, 0:OWN]),
             "fa": fa, "fb": fb}
        d.update(shared)
        in_maps.append(d)
    return in_maps


class _SpmdRunner:
    def __init__(self, nc, n_cores=NC):
        import jax
        from jax.sharding import Mesh, PartitionSpec
        from jax.experimental.shard_map import shard_map
        import concourse.mybir as mybir
        from concourse import bass2jax
        bass2jax.install_neuronx_cc_hook()
        self.jax = jax
        self.n_cores = n_cores
        partition_name = (
            nc.partition_id_tensor.name if nc.partition_id_tensor else None)
        in_names, out_names, out_avals = [], [], []
        for alloc in nc.m.functions[0].allocations:
            if not isinstance(alloc, mybir.MemoryLocationSet):
                continue
            name = alloc.memorylocations[0].name
            if alloc.kind == "ExternalInput":
                if name != partition_name:
                    in_names.append(name)
            elif alloc.kind == "ExternalOutput":
                out_names.append(name)
                out_avals.append(jax.core.ShapedArray(
                    tuple(alloc.tensor_shape), mybir.dt.np(alloc.dtype)))
        self.in_names = in_names
        self.out_names = out_names
        self.out_avals = out_avals
        all_in = in_names + out_names
        if partition_name is not None:
            all_in.append(partition_name)

        def _body(*args):
            operands = list(args)
            if partition_name is not None:
                operands.append(bass2jax.partition_id_tensor())
            outs = bass2jax._bass_exec_p.bind(
                *operands, out_avals=tuple(out_avals),
                in_names=tuple(all_in), out_names=tuple(out_names),
                lowering_input_output_aliases=(),
                sim_require_finite=True, sim_require_nnan=True, nc=nc)
            return tuple(outs)

        devices = jax.devices()[:n_cores]
        self.mesh = Mesh(np.asarray(devices), ("core",))
        n_io = len(in_names) + len(out_names)
        self.fn = jax.jit(
            shard_map(_body, mesh=self.mesh,
                      in_specs=(PartitionSpec("core"),) * n_io,
                      out_specs=(PartitionSpec("core"),) * len(out_names),
                      check_rep=False),
            keep_unused=True)
        self._dev_in = None

    def put_inputs(self, in_maps):
        from jax.sharding import NamedSharding, PartitionSpec
        jax = self.jax
        sh = NamedSharding(self.mesh, PartitionSpec("core"))
        concat = []
        for name in self.in_names:
            arrs = [np.asarray(in_maps[c][name]) for c in range(self.n_cores)]
            concat.append(jax.device_put(np.concatenate(arrs, axis=0), sh))
        for av in self.out_avals:
            z = np.zeros((self.n_cores * av.shape[0], *av.shape[1:]), av.dtype)
            concat.append(jax.device_put(z, sh))
        self._dev_in = concat

    def run(self):
        jax = self.jax
        outs = self.fn(*self._dev_in)
        jax.block_until_ready(outs)
        results = []
        for c in range(self.n_cores):
            d = {}
            for i, name in enumerate(self.out_names):
                av = self.out_avals[i]
                d[name] = np.asarray(outs[i]).reshape(
                    self.n_cores, *av.shape)[c]
            results.append(d)
        return results

    def time_exec(self, warmup=3, m1=4, m2=12, reps=3, trials=6):
        """Estimate per-call device time by differencing burst timings,
        which cancels the constant dispatch/RTT overhead of the axon
        tunnel."""
        import time
        jax = self.jax
        for _ in range(warmup):
            jax.block_until_ready(self.fn(*self._dev_in))

        def burst(m):
            t0 = time.perf_counter()
            outs = None
            for _ in range(m):
                outs = self.fn(*self._dev_in)
            jax.block_until_ready(outs)
            return time.perf_counter() - t0

        t1s, t2s = [], []
        for _ in range(trials):
            for _ in range(reps):
                t1s.append(burst(m1))
                t2s.append(burst(m2))
        return (min(t2s) - min(t1s)) / (m2 - m1)


def _get_runner():
    if "runner" not in _STATE:
        nc = _build_program()
        _STATE["runner"] = _SpmdRunner(nc)
    return _STATE["runner"]


def kernel(x, W_attn, W_proj, b_proj, W_fc1, b_fc1, W_fc2, b_fc2,
           ln1_g, ln1_b, ln2_g, ln2_b):
    runner = _get_runner()
    in_maps = _prep_in_maps(x, W_attn, W_proj, b_proj, W_fc1, b_fc1,
                            W_fc2, b_fc2, ln1_g, ln1_b, ln2_g, ln2_b)
    runner.put_inputs(in_maps)
    results = runner.run()
    out = np.empty((B, T, C), np.float32)
    for c in range(NC):
        b, r = c // 2, c % 2
        ot = results[c]["out"]                # [128, NKC, OWN]
        feat = ot.transpose(1, 0, 2).reshape(C, OWN)
        out[b, _perm(r)[0:OWN], :] = feat.T
    return out


# revision 4
# speedup vs baseline: 1.0517x; 1.0517x over previous
"""Dense transformer block (B=4, T=2048, C=1024, H=16, FF=4096) on 8
Trainium2 NeuronCores — v2.

Sharding: sequence-parallel, zero collectives. Core c handles batch
b = c // 2 and query-token half r = c % 2. The host permutes each
core's token order so its OWN 1024 query tokens are always columns
[0:1024) (zigzag: r=0 owns global [0:512)+[1536:2048), r=1 owns
[512:1536)). K/V/LN1 are computed redundantly for the full permuted
sequence, entirely in SBUF (no DRAM bounces).

All matmul operands are bf16 (fp32 PSUM accumulate); LN statistics in
fp32. LN affine params are folded into the projection weights host-side
(the K-side bias is dropped entirely: a per-query constant logit shift
cancels in softmax). Causality:
  - 8 triangular chunk-slots use bf16 masks multiplied post-exp (DVE)
  - core-dependent all-zero/all-one chunk-slots use pre-flagged V
    copies (vt2/vt3), so no mask work at all
K-projection is interleaved per-head-pair with attention so the PE
stays busy during the exp-bound softmax stream.
"""
import numpy as np
import ml_dtypes

B, T, C = 4, 2048, 1024
H, D, FF = 16, 64, 4096
NC = 8
NKC = C // 128     # 8 feature chunks
NFFC = FF // 128   # 32
OWN = 1024
EPS = 1e-5

_STATE = {}


def _build_program():
    import concourse.bacc as bacc
    import concourse.mybir as mybir
    from concourse.tile import TileContext

    F32R = mybir.dt.float32r
    F32 = mybir.dt.float32
    BF16 = mybir.dt.bfloat16
    AF = mybir.ActivationFunctionType
    OP = mybir.AluOpType

    nc = bacc.Bacc("TRN2", target_bir_lowering=False, debug=False,
                   num_devices=NC)

    xbf_d = nc.dram_tensor("xbf", [128, NKC, T], BF16, kind="ExternalInput")
    xq_d = nc.dram_tensor("xq", [128, NKC, OWN], F32, kind="ExternalInput")
    wq_d = nc.dram_tensor("wq", [8, 128, NKC, 128], BF16, kind="ExternalInput")
    wk_d = nc.dram_tensor("wk", [8, 128, NKC, 128], BF16, kind="ExternalInput")
    wv_d = nc.dram_tensor("wv", [128, NKC, C], BF16, kind="ExternalInput")
    wp_d = nc.dram_tensor("wp", [8, 128, NKC, 128], BF16, kind="ExternalInput")
    wf1_d = nc.dram_tensor("wf1", [NFFC, 128, NKC, 128], BF16,
                           kind="ExternalInput")
    wf2_d = nc.dram_tensor("wf2", [NKC, 128, NFFC, 128], BF16,
                           kind="ExternalInput")
    qb_d = nc.dram_tensor("qb", [128, 8], F32, kind="ExternalInput")
    bp_d = nc.dram_tensor("bp", [128, NKC], F32, kind="ExternalInput")
    bf1_d = nc.dram_tensor("bf1", [128, NFFC], F32, kind="ExternalInput")
    bf2_d = nc.dram_tensor("bf2", [128, NKC], F32, kind="ExternalInput")
    # 4 triangular mask slots (same pattern set on every core)
    mask_d = nc.dram_tensor("masks", [128, 4, 512], BF16,
                            kind="ExternalInput")
    fa_d = nc.dram_tensor("fa", [128, 64], F32, kind="ExternalInput")
    fb_d = nc.dram_tensor("fb", [128, 64], F32, kind="ExternalInput")
    out_d = nc.dram_tensor("out", [128, NKC, OWN], F32, kind="ExternalOutput")

    def mm(ps, lhsT, rhs, start, stop):
        nc.tensor.matmul(ps, lhsT, rhs, start=start, stop=stop)

    with TileContext(nc, pool_alloc_mode="queue") as tc:
        consts_cm = tc.tile_pool(name="consts", bufs=1)
        consts = consts_cm.__enter__()

        ones128 = consts.tile([128, 1], F32R)
        nc.vector.memset(ones128.bitcast(F32), 1.0)
        ones128b = consts.tile([128, 1], BF16)
        nc.vector.memset(ones128b, 1.0)
        onesrow = consts.tile([1, 128], F32R)
        nc.vector.memset(onesrow.bitcast(F32), 1.0)
        negrow = consts.tile([1, 128], F32R)
        nc.vector.memset(negrow.bitcast(F32), -1.0)
        eps_t = consts.tile([1, 1], F32)
        nc.vector.memset(eps_t, EPS)
        qb_t = consts.tile([128, 8], F32)
        bp_t = consts.tile([128, NKC], F32)
        bf1_t = consts.tile([128, NFFC], F32)
        bf2_t = consts.tile([128, NKC], F32)
        fa_t = consts.tile([128, 64], F32)
        fb_t = consts.tile([128, 64], F32)

        # x2t/yt outlive the attention pools -> their pools open first
        x2_cm = tc.tile_pool(name="x2p", bufs=1)
        x2p = x2_cm.__enter__()
        x2t = x2p.tile([128, NKC, OWN], F32R)
        yt_cm = tc.tile_pool(name="ytp", bufs=1)
        ytp = yt_cm.__enter__()
        yt = ytp.tile([128, NKC, OWN], BF16)

        # attention-lifetime activation tiles (closed after proj)
        pers_cm = tc.tile_pool(name="pers", bufs=1)
        pers = pers_cm.__enter__()
        lnx = [pers.tile([128, NKC, 512], BF16, tag=f"lnx{tb}",
                         name=f"lnx{tb}") for tb in range(4)]
        vt = [pers.tile([128, 16, 8, 65], BF16, tag=f"vt{g}",
                        name=f"vt{g}") for g in range(2)]
        vt2 = pers.tile([128, 4, 16, 65], BF16, tag="vt2")
        vt3 = pers.tile([128, 4, 16, 65], BF16, tag="vt3")
        qts = [pers.tile([128, OWN], BF16, tag=f"q{hp}", name=f"q{hp}")
               for hp in range(8)]

        # ------- Phases A-C interleaved: LN1 + V + Q per token-block -------
        xtb_cm = tc.tile_pool(name="xtbp", bufs=2)
        xtbp = xtb_cm.__enter__()
        lw_cm = tc.tile_pool(name="lnw", bufs=2)
        lnw = lw_cm.__enter__()
        lst_cm = tc.tile_pool(name="lnst", bufs=1)
        lst = lst_cm.__enter__()
        wvp_cm = tc.tile_pool(name="wvp", bufs=1)
        wvp = wvp_cm.__enter__()
        wv_t = wvp.tile([128, NKC, C], BF16)
        qw_cm = tc.tile_pool(name="qw", bufs=3)
        qw = qw_cm.__enter__()
        lps_cm = tc.tile_pool(name="lnps", bufs=1, space="PSUM")
        lps = lps_cm.__enter__()
        pps_cm = tc.tile_pool(name="projps", bufs=2, space="PSUM")
        pjps = pps_cm.__enter__()

        nc.vector.memset(vt[0][:, :, :, 64:65], 1.0)
        nc.vector.memset(vt[1][:, :, :, 64:65], 1.0)
        def ln1_tb(tb):
            sl = slice(tb * 512, (tb + 1) * 512)
            xtb = xtbp.tile([128, NKC, 512], BF16, tag="xtb")
            nc.sync.dma_start(out=xtb, in_=xbf_d[:, :, sl])
            ps_s = lps.tile([1, 512], F32, tag="ps_s")
            for k in range(NKC):
                mm(ps_s, ones128b, xtb[:, k, :], k == 0, k == NKC - 1)
            sqs = []
            for k in range(NKC):
                sq = lnw.tile([128, 512], BF16, tag=f"sq{k % 2}")
                nc.scalar.activation(out=sq, in_=xtb[:, k, :], func=AF.Square)
                sqs.append(sq)
            ps_q = lps.tile([1, 512], F32, tag="ps_q")
            for k in range(NKC):
                mm(ps_q, ones128b, sqs[k], k == 0, k == NKC - 1)
            mu = lst.tile([1, 512], F32R, tag="mu")
            nc.vector.tensor_scalar_mul(out=mu, in0=ps_s, scalar1=1.0 / C)
            msq = lst.tile([1, 512], F32, tag="msq")
            nc.vector.tensor_scalar_mul(out=msq, in0=ps_q, scalar1=1.0 / C)
            var = lst.tile([1, 512], F32, tag="var")
            nc.vector.tensor_mul(out=var, in0=mu.bitcast(F32),
                                 in1=mu.bitcast(F32))
            nc.vector.tensor_sub(out=var, in0=msq, in1=var)
            nc.scalar.activation(out=msq, in_=var, func=AF.Sqrt,
                                 bias=eps_t, scale=1.0)
            rstd = lst.tile([1, 512], F32R, tag="rstd")
            with nc.allow_low_precision(reason="f32r rstd"):
                nc.vector.reciprocal(out=rstd, in_=msq)
            pmr = lst.tile([1, 512], F32R, tag="pmr")
            with nc.allow_low_precision(reason="f32r pmr"):
                nc.vector.tensor_mul(out=pmr, in0=mu.bitcast(F32),
                                     in1=rstd.bitcast(F32))
            ps_r = lps.tile([128, 512], F32, tag="ps_r")
            mm(ps_r, onesrow, rstd, True, True)
            ps_n = lps.tile([128, 512], F32, tag="ps_n")
            mm(ps_n, negrow, pmr, True, True)
            rb_s = lnw.tile([128, 512], BF16, tag="rb_s")
            nc.scalar.activation(out=rb_s, in_=ps_r, func=AF.Copy)
            nb_s = lnw.tile([128, 512], BF16, tag="nb_s")
            nc.scalar.activation(out=nb_s, in_=ps_n, func=AF.Copy)
            for k in range(NKC):
                nc.vector.tensor_mul(out=lnx[tb][:, k, :], in0=xtb[:, k, :],
                                     in1=rb_s)
                nc.vector.tensor_add(out=lnx[tb][:, k, :],
                                     in0=lnx[tb][:, k, :], in1=nb_s)

        def v_chunk(cch):
            tbi, coff = cch // 4, (cch % 4) * 128
            ps_v = pjps.tile([128, C], F32, tag="pv")
            for h2 in range(2):
                vsl = slice(h2 * 512, (h2 + 1) * 512)
                for k in range(NKC):
                    mm(ps_v[:, vsl], lnx[tbi][:, k, coff:coff + 128],
                       wv_t[:, k, vsl], k == 0, k == NKC - 1)
            for g in range(2):
                nc.vector.tensor_copy(
                    out=vt[g][:, cch, :, 0:64],
                    in_=ps_v[:, g * 512:(g + 1) * 512]
                    .rearrange("p (h d) -> p h d", h=8))
            if 8 <= cch < 12:
                nc.vector.tensor_scalar_mul(
                    out=vt2[:, cch - 8, :, 0:64],
                    in0=ps_v.rearrange("p (h d) -> p h d", h=16),
                    scalar1=fa_t[:, 0:1])
            if 12 <= cch:
                nc.vector.tensor_scalar_mul(
                    out=vt3[:, cch - 12, :, 0:64],
                    in0=ps_v.rearrange("p (h d) -> p h d", h=16),
                    scalar1=fb_t[:, 0:1])

        def q_hp(hp):
            wt = qw.tile([128, NKC, 128], BF16, tag="w")
            nc.sync.dma_start(out=wt, in_=wq_d[hp])
            ps = pjps.tile([128, OWN], F32, tag="pv")
            for tb in range(2):
                sl = slice(tb * 512, (tb + 1) * 512)
                for k in range(NKC):
                    mm(ps[:, sl], wt[:, k, :], lnx[tb][:, k, :],
                       k == 0, k == NKC - 1)
            nc.vector.tensor_scalar_add(out=qts[hp], in0=ps,
                                        scalar1=qb_t[:, hp:hp + 1])

        for tb in range(4):
            ln1_tb(tb)
            if tb == 0:
                nc.sync.dma_start(out=wv_t, in_=wv_d[:, :, :])
                nc.sync.dma_start(out=fa_t, in_=fa_d[:, :])
                nc.sync.dma_start(out=fb_t, in_=fb_d[:, :])
                nc.sync.dma_start(out=qb_t, in_=qb_d[:, :])
                nc.sync.dma_start(out=bp_t, in_=bp_d[:, :])
                nc.sync.dma_start(out=bf1_t, in_=bf1_d[:, :])
                nc.sync.dma_start(out=bf2_t, in_=bf2_d[:, :])
                nc.gpsimd.tensor_copy(
                    out=vt2[:, :, :, 64:65]
                    .rearrange("p a h o -> p (a h o)"), in_=fa_t)
                nc.gpsimd.tensor_copy(
                    out=vt3[:, :, :, 64:65]
                    .rearrange("p a h o -> p (a h o)"), in_=fb_t)
            if tb >= 1:
                for cch in range(4 * (tb - 1), 4 * tb):
                    v_chunk(cch)
            if tb == 2:
                for hp in range(4):
                    q_hp(hp)
            if tb == 3:
                for hp in range(4, 8):
                    q_hp(hp)
        for cch in range(12, 16):
            v_chunk(cch)

        pps_cm.__exit__(None, None, None)
        lps_cm.__exit__(None, None, None)
        qw_cm.__exit__(None, None, None)
        wvp_cm.__exit__(None, None, None)
        lst_cm.__exit__(None, None, None)
        lw_cm.__exit__(None, None, None)
        xtb_cm.__exit__(None, None, None)

        # ---------- Phase D: K projection + attention, per head-pair ----------
        # qb0 slots: chunks 0..3 (tri, mask slot=ci), 8..11 (flagged vt2)
        # qb1 slots: chunks 0..3, 8..11 (plain), 4..7 (tri, slot=ci-4),
        #            12..15 (flagged vt3)
        SLOTS = {
            0: [(ci, "tri", ci) for ci in range(4)]
               + [(ci, "v2", None) for ci in range(8, 12)],
            1: [(ci, "one", None) for ci in range(4)]
               + [(ci, "tri", ci - 4) for ci in range(4, 8)]
               + [(ci, "one", None) for ci in range(8, 12)]
               + [(ci, "v3", None) for ci in range(12, 16)],
        }

        kw_cm = tc.tile_pool(name="kw", bufs=3)
        kw = kw_cm.__enter__()
        mask_t = kw.tile([128, 4, 512], BF16, tag="mask", bufs=1)
        nc.sync.dma_start(out=mask_t, in_=mask_d[:, :, :])
        kt_cm = tc.tile_pool(name="ktp", bufs=3)
        ktp = kt_cm.__enter__()
        att_cm = tc.tile_pool(name="attw", bufs=4)
        attw = att_cm.__enter__()
        sm_cm = tc.tile_pool(name="smw", bufs=2)
        smw = sm_cm.__enter__()
        kps_cm = tc.tile_pool(name="kps", bufs=2, space="PSUM")
        kps = kps_cm.__enter__()
        sps_cm = tc.tile_pool(name="sps", bufs=2, space="PSUM")
        sps = sps_cm.__enter__()
        yps_cm = tc.tile_pool(name="yps", bufs=1, space="PSUM")
        yps = yps_cm.__enter__()

        for hp in range(8):
            ha, hb = 2 * hp, 2 * hp + 1
            g = hp // 4
            wt = kw.tile([128, NKC, 128], BF16, tag="w")
            nc.sync.dma_start(out=wt, in_=wk_d[hp])
            kt_t = ktp.tile([128, T], BF16, tag="kt")
            for tb in range(4):
                sl = slice(tb * 512, (tb + 1) * 512)
                ps_k = kps.tile([128, 512], F32, tag="mm")
                for k in range(NKC):
                    mm(ps_k, wt[:, k, :], lnx[tb][:, k, :],
                       k == 0, k == NKC - 1)
                nc.vector.tensor_copy(out=kt_t[:, sl], in_=ps_k)

            for qb in range(2):
                slots = SLOTS[qb]
                nsl = len(slots)
                qsl = slice(qb * 512, (qb + 1) * 512)
                ps_y = yps.tile([65, 2, 512], F32, tag="y")
                for idx, (ci, kind, mslot) in enumerate(slots):
                    csl = slice(ci * 128, (ci + 1) * 128)
                    ps_s = sps.tile([128, 2, 512], F32, tag="s")
                    mm(ps_s[:, 0, :], kt_t[0:64, csl], qts[hp][0:64, qsl],
                       True, True)
                    mm(ps_s[:, 1, :], kt_t[64:128, csl], qts[hp][64:128, qsl],
                       True, True)
                    ptm = attw.tile([128, 2, 512], BF16, tag="ptm")
                    if kind == "tri":
                        pt = attw.tile([128, 2, 512], BF16, tag="pt")
                        nc.scalar.activation(out=pt, in_=ps_s, func=AF.Exp)
                        nc.vector.tensor_mul(
                            out=ptm, in0=pt,
                            in1=mask_t[:, mslot:mslot + 1, :]
                            .broadcast_to([128, 2, 512]))
                    else:
                        nc.scalar.activation(out=ptm, in_=ps_s, func=AF.Exp)
                    if kind == "v2":
                        va, vb = vt2[:, ci - 8, ha, :], vt2[:, ci - 8, hb, :]
                    elif kind == "v3":
                        va, vb = vt3[:, ci - 12, ha, :], vt3[:, ci - 12, hb, :]
                    else:
                        va = vt[g][:, ci, ha - g * 8, :]
                        vb = vt[g][:, ci, hb - g * 8, :]
                    mm(ps_y[:, 0, :], va, ptm[:, 0, :], idx == 0,
                       idx == nsl - 1)
                    mm(ps_y[:, 1, :], vb, ptm[:, 1, :], idx == 0,
                       idx == nsl - 1)
                yraw = smw.tile([65, 2, 512], F32, tag="yraw")
                nc.vector.tensor_copy(out=yraw, in_=ps_y)
                for hh, h in ((0, ha), (1, hb)):
                    rd = smw.tile([1, 512], F32R, tag="rd")
                    with nc.allow_low_precision(reason="softmax denom"):
                        nc.vector.reciprocal(out=rd, in_=yraw[64:65, hh, :])
                    rb = smw.tile([64, 512], F32, tag="rb")
                    nc.gpsimd.partition_broadcast(rb, rd.bitcast(F32))
                    nc.vector.tensor_mul(
                        out=yt[64 * hh:64 * hh + 64, hp, qsl],
                        in0=yraw[0:64, hh, :], in1=rb)

        yps_cm.__exit__(None, None, None)
        sps_cm.__exit__(None, None, None)
        kps_cm.__exit__(None, None, None)
        sm_cm.__exit__(None, None, None)
        att_cm.__exit__(None, None, None)
        kt_cm.__exit__(None, None, None)
        kw_cm.__exit__(None, None, None)
        pers_cm.__exit__(None, None, None)

        # ------- Phases E/F/G interleaved over 512-token halves -------
        l2x_cm = tc.tile_pool(name="l2x", bufs=1)
        l2xp = l2x_cm.__enter__()
        ln2x = l2xp.tile([128, NKC, OWN], BF16)
        m1_cm = tc.tile_pool(name="m1p", bufs=1)
        m1p = m1_cm.__enter__()
        m1t = m1p.tile([128, NFFC, OWN], BF16)
        mw_cm = tc.tile_pool(name="mw", bufs=3)
        mw = mw_cm.__enter__()
        mo_cm = tc.tile_pool(name="mo", bufs=2)
        mo = mo_cm.__enter__()
        mps_cm = tc.tile_pool(name="mps", bufs=2, space="PSUM")
        mps = mps_cm.__enter__()
        pw_cm = tc.tile_pool(name="pw", bufs=3)
        pw = pw_cm.__enter__()
        l2w_cm = tc.tile_pool(name="l2w", bufs=2)
        l2w = l2w_cm.__enter__()
        l2ps_cm = tc.tile_pool(name="l2ps", bufs=1, space="PSUM")
        l2ps = l2ps_cm.__enter__()

        def proj_half(h2):
            tsl = slice(h2 * 512, (h2 + 1) * 512)
            for oc in range(8):
                wt = pw.tile([128, NKC, 128], BF16, tag="w")
                nc.sync.dma_start(out=wt, in_=wp_d[oc])
                xqo = pw.tile([128, 512], F32, tag="xq")
                nc.sync.dma_start(out=xqo, in_=xq_d[:, oc, tsl])
                ps = mps.tile([128, 512], F32, tag="mm1")
                for k in range(NKC):
                    mm(ps, wt[:, k, :], yt[:, k, tsl], k == 0, k == NKC - 1)
                with nc.allow_low_precision(reason="f32r x2"):
                    nc.vector.scalar_tensor_tensor(
                        out=x2t[:, oc, tsl], in0=ps,
                        scalar=bp_t[:, oc:oc + 1],
                        in1=xqo, op0=OP.add, op1=OP.add)

        def ln2_half(h2):
            tsl = slice(h2 * 512, (h2 + 1) * 512)
            ps_s2 = l2ps.tile([1, 512], F32, tag="ps_s")
            for k in range(NKC):
                mm(ps_s2, ones128, x2t[:, k, tsl], k == 0, k == NKC - 1)
            sq2s = []
            for k in range(NKC):
                sq = l2w.tile([128, 512], BF16, tag=f"sq{k % 2}")
                nc.scalar.activation(out=sq, in_=x2t[:, k, tsl].bitcast(F32),
                                     func=AF.Square)
                sq2s.append(sq)
            ps_q2 = l2ps.tile([1, 512], F32, tag="ps_q")
            for k in range(NKC):
                mm(ps_q2, ones128b, sq2s[k], k == 0, k == NKC - 1)
            mu2_ = l2w.tile([1, 512], F32R, tag="mu")
            nc.vector.tensor_scalar_mul(out=mu2_, in0=ps_s2, scalar1=1.0 / C)
            msq2 = l2w.tile([1, 512], F32, tag="msq")
            nc.vector.tensor_scalar_mul(out=msq2, in0=ps_q2, scalar1=1.0 / C)
            var2 = l2w.tile([1, 512], F32, tag="var")
            nc.vector.tensor_mul(out=var2, in0=mu2_.bitcast(F32),
                                 in1=mu2_.bitcast(F32))
            nc.vector.tensor_sub(out=var2, in0=msq2, in1=var2)
            nc.scalar.activation(out=msq2, in_=var2, func=AF.Sqrt,
                                 bias=eps_t, scale=1.0)
            rstd2 = l2w.tile([1, 512], F32R, tag="rstd")
            with nc.allow_low_precision(reason="f32r rstd"):
                nc.vector.reciprocal(out=rstd2, in_=msq2)
            pmr2 = l2w.tile([1, 512], F32R, tag="pmr")
            with nc.allow_low_precision(reason="f32r pmr"):
                nc.vector.tensor_mul(out=pmr2, in0=mu2_.bitcast(F32),
                                     in1=rstd2.bitcast(F32))
            ps_r2 = l2ps.tile([128, 512], F32, tag="ps_r")
            mm(ps_r2, onesrow, rstd2, True, True)
            ps_n2 = l2ps.tile([128, 512], F32, tag="ps_n")
            mm(ps_n2, negrow, pmr2, True, True)
            nb2 = l2w.tile([128, 512], BF16, tag="nb2")
            nc.scalar.activation(out=nb2, in_=ps_n2, func=AF.Copy)
            for k in range(NKC):
                nc.vector.tensor_mul(out=ln2x[:, k, tsl],
                                     in0=x2t[:, k, tsl].bitcast(F32),
                                     in1=ps_r2)
                nc.vector.tensor_add(out=ln2x[:, k, tsl],
                                     in0=ln2x[:, k, tsl], in1=nb2)

        def fc1_half(h2):
            tsl = slice(h2 * 512, (h2 + 1) * 512)
            for ffc in range(NFFC):
                wt = mw.tile([128, NKC, 128], BF16, tag="w1")
                nc.sync.dma_start(out=wt, in_=wf1_d[ffc])
                ps = mps.tile([128, 512], F32, tag="mm1")
                for k in range(NKC):
                    mm(ps, wt[:, k, :], ln2x[:, k, tsl], k == 0, k == NKC - 1)
                nc.scalar.activation(out=m1t[:, ffc, tsl], in_=ps,
                                     func=AF.Relu,
                                     bias=bf1_t[:, ffc:ffc + 1], scale=1.0)

        proj_half(0)
        ln2_half(0)
        proj_half(1)
        fc1_half(0)
        ln2_half(1)
        fc1_half(1)

        for oc in range(NKC):
            wt2 = mw.tile([128, NFFC, 128], BF16, tag="w2")
            nc.sync.dma_start(out=wt2, in_=wf2_d[oc])
            ot = mo.tile([128, OWN], F32, tag="ot")
            for h2 in range(2):
                tsl = slice(h2 * 512, (h2 + 1) * 512)
                ps = mps.tile([128, 512], F32, tag="mm2")
                for k in range(NFFC):
                    mm(ps, wt2[:, k, :], m1t[:, k, tsl], k == 0, k == NFFC - 1)
                nc.vector.scalar_tensor_tensor(
                    out=ot[:, tsl], in0=ps, scalar=bf2_t[:, oc:oc + 1],
                    in1=x2t[:, oc, tsl].bitcast(F32), op0=OP.add, op1=OP.add)
            nc.sync.dma_start(out=out_d[:, oc, :], in_=ot)

        l2ps_cm.__exit__(None, None, None)
        l2w_cm.__exit__(None, None, None)
        pw_cm.__exit__(None, None, None)
        mps_cm.__exit__(None, None, None)
        mo_cm.__exit__(None, None, None)
        mw_cm.__exit__(None, None, None)
        m1_cm.__exit__(None, None, None)
        l2x_cm.__exit__(None, None, None)
        yt_cm.__exit__(None, None, None)
        x2_cm.__exit__(None, None, None)
        consts_cm.__exit__(None, None, None)

    nc.compile()
    return nc


def _perm(r):
    if r == 0:
        return np.concatenate([np.arange(0, 512), np.arange(1536, 2048),
                               np.arange(512, 1536)])
    return np.concatenate([np.arange(512, 1536), np.arange(0, 512),
                           np.arange(1536, 2048)])


def _prep_in_maps(x, W_attn, W_proj, b_proj, W_fc1, b_fc1, W_fc2, b_fc2,
                  ln1_g, ln1_b, ln2_g, ln2_b):
    f32 = np.float32
    bf16 = ml_dtypes.bfloat16
    x = np.asarray(x, f32)
    W_attn = np.asarray(W_attn, f32)
    Wq, Wk, Wv = W_attn[:, 0:C], W_attn[:, C:2 * C], W_attn[:, 2 * C:3 * C]
    W_proj = np.asarray(W_proj, f32)
    W_fc1 = np.asarray(W_fc1, f32)
    W_fc2 = np.asarray(W_fc2, f32)
    g1 = np.asarray(ln1_g, f32)
    b1 = np.asarray(ln1_b, f32)
    g2 = np.asarray(ln2_g, f32)
    b2 = np.asarray(ln2_b, f32)

    s = 1.0 / np.sqrt(D)
    Wq_f = (g1[:, None] * Wq) * s
    Wk_f = g1[:, None] * Wk
    Wv_f = g1[:, None] * Wv
    Wf1_f = g2[:, None] * W_fc1
    qbias = (b1 @ Wq) * s                      # [C]
    vbias = b1 @ Wv                            # [C]
    bp_f = np.asarray(b_proj, f32) + vbias @ W_proj
    bf1_f = np.asarray(b_fc1, f32) + b2 @ W_fc1
    bf2_f = np.asarray(b_fc2, f32)

    def lhs_tiles(W, nout):
        nin = W.shape[0] // 128
        return np.ascontiguousarray(
            W.reshape(nin, 128, nout, 128).transpose(2, 1, 0, 3)
        ).astype(bf16)

    def vec(v, nk):
        return np.ascontiguousarray(np.asarray(v, f32).reshape(nk, 128).T)

    kvp = np.arange(128)
    qi = np.arange(512)
    masks = np.zeros((128, 4, 512), np.float32)
    for j in range(4):
        masks[:, j, :] = (128 * j + kvp[:, None]) <= qi[None, :]

    shared = {
        "wq": lhs_tiles(Wq_f, 8), "wk": lhs_tiles(Wk_f, 8),
        "wv": np.ascontiguousarray(
            Wv_f.reshape(NKC, 128, C).transpose(1, 0, 2)).astype(bf16),
        "wp": lhs_tiles(W_proj, 8),
        "wf1": lhs_tiles(Wf1_f, NFFC), "wf2": lhs_tiles(W_fc2, NKC),
        "qb": vec(qbias, 8), "bp": vec(bp_f, NKC),
        "bf1": vec(bf1_f, NFFC), "bf2": vec(bf2_f, NKC),
        "masks": masks.astype(bf16),
    }

    in_maps = []
    for c in range(NC):
        b, r = c // 2, c % 2
        perm = _perm(r)
        xs = x[b][perm]                       # [T, C] permuted
        xt = np.ascontiguousarray(
            xs.T.reshape(NKC, 128, T).transpose(1, 0, 2))
        fa = np.full((128, 64), float(r == 1), np.float32)
        fb = np.full((128, 64), float(r == 0), np.float32)
        d = {"xbf": xt.astype(bf16), "xq": np.ascontiguousarray(xt[:, :, 0:OWN]),
             "fa": fa, "fb": fb}
        d.update(shared)
        in_maps.append(d)
    return in_maps


class _SpmdRunner:
    def __init__(self, nc, n_cores=NC):
        import jax
        from jax.sharding import Mesh, PartitionSpec
        from jax.experimental.shard_map import shard_map
        import concourse.mybir as mybir
        from concourse import bass2jax
        bass2jax.install_neuronx_cc_hook()
        self.jax = jax
        self.n_cores = n_cores
        partition_name = (
            nc.partition_id_tensor.name if nc.partition_id_tensor else None)
        in_names, out_names, out_avals = [], [], []
        for alloc in nc.m.functions[0].allocations:
            if not isinstance(alloc, mybir.MemoryLocationSet):
                continue
            name = alloc.memorylocations[0].name
            if alloc.kind == "ExternalInput":
                if name != partition_name:
                    in_names.append(name)
            elif alloc.kind == "ExternalOutput":
                out_names.append(name)
                out_avals.append(jax.core.ShapedArray(
                    tuple(alloc.tensor_shape), mybir.dt.np(alloc.dtype)))
        self.in_names = in_names
        self.out_names = out_names
        self.out_avals = out_avals
        all_in = in_names + out_names
        if partition_name is not None:
            all_in.append(partition_name)

        def _body(*args):
            operands = list(args)
            if partition_name is not None:
                operands.append(bass2jax.partition_id_tensor())
            outs = bass2jax._bass_exec_p.bind(
                *operands, out_avals=tuple(out_avals),
                in_names=tuple(all_in), out_names=tuple(out_names),
                lowering_input_output_aliases=(),
                sim_require_finite=True, sim_require_nnan=True, nc=nc)
            return tuple(outs)

        devices = jax.devices()[:n_cores]
        self.mesh = Mesh(np.asarray(devices), ("core",))
        n_io = len(in_names) + len(out_names)
        self.fn = jax.jit(
            shard_map(_body, mesh=self.mesh,
                      in_specs=(PartitionSpec("core"),) * n_io,
                      out_specs=(PartitionSpec("core"),) * len(out_names),
                      check_rep=False),
            keep_unused=True)
        self._dev_in = None

    def put_inputs(self, in_maps):
        from jax.sharding import NamedSharding, PartitionSpec
        jax = self.jax
        sh = NamedSharding(self.mesh, PartitionSpec("core"))
        concat = []
        for name in self.in_names:
            arrs = [np.asarray(in_maps[c][name]) for c in range(self.n_cores)]
            concat.append(jax.device_put(np.concatenate(arrs, axis=0), sh))
        for av in self.out_avals:
            z = np.zeros((self.n_cores * av.shape[0], *av.shape[1:]), av.dtype)
            concat.append(jax.device_put(z, sh))
        self._dev_in = concat

    def run(self):
        jax = self.jax
        outs = self.fn(*self._dev_in)
        jax.block_until_ready(outs)
        results = []
        for c in range(self.n_cores):
            d = {}
            for i, name in enumerate(self.out_names):
                av = self.out_avals[i]
                d[name] = np.asarray(outs[i]).reshape(
                    self.n_cores, *av.shape)[c]
            results.append(d)
        return results

    def time_exec(self, warmup=3, m1=4, m2=12, reps=3, trials=6):
        """Estimate per-call device time by differencing burst timings,
        which cancels the constant dispatch/RTT overhead of the axon
        tunnel."""
        import time
        jax = self.jax
        for _ in range(warmup):
            jax.block_until_ready(self.fn(*self._dev_in))

        def burst(m):
            t0 = time.perf_counter()
            outs = None
            for _ in range(m):
                outs = self.fn(*self._dev_in)
            jax.block_until_ready(outs)
            return time.perf_counter() - t0

        t1s, t2s = [], []
        for _ in range(trials):
            for _ in range(reps):
                t1s.append(burst(m1))
                t2s.append(burst(m2))
        return (min(t2s) - min(t1s)) / (m2 - m1)


def _get_runner():
    if "runner" not in _STATE:
        nc = _build_program()
        _STATE["runner"] = _SpmdRunner(nc)
    return _STATE["runner"]


def kernel(x, W_attn, W_proj, b_proj, W_fc1, b_fc1, W_fc2, b_fc2,
           ln1_g, ln1_b, ln2_g, ln2_b):
    runner = _get_runner()
    in_maps = _prep_in_maps(x, W_attn, W_proj, b_proj, W_fc1, b_fc1,
                            W_fc2, b_fc2, ln1_g, ln1_b, ln2_g, ln2_b)
    runner.put_inputs(in_maps)
    results = runner.run()
    out = np.empty((B, T, C), np.float32)
    for c in range(NC):
        b, r = c // 2, c % 2
        ot = results[c]["out"]                # [128, NKC, OWN]
        feat = ot.transpose(1, 0, 2).reshape(C, OWN)
        out[b, _perm(r)[0:OWN], :] = feat.T
    return out


# revision 5
# speedup vs baseline: 1.6775x; 1.5951x over previous
"""Dense transformer block (B=4, T=2048, C=1024, H=16, FF=4096) on 8
Trainium2 NeuronCores — v2.

Sharding: sequence-parallel, zero collectives. Core c handles batch
b = c // 2 and query-token half r = c % 2. The host permutes each
core's token order so its OWN 1024 query tokens are always columns
[0:1024) (zigzag: r=0 owns global [0:512)+[1536:2048), r=1 owns
[512:1536)). K/V/LN1 are computed redundantly for the full permuted
sequence, entirely in SBUF (no DRAM bounces).

All matmul operands are bf16 (fp32 PSUM accumulate); LN statistics in
fp32. LN affine params are folded into the projection weights host-side
(the K-side bias is dropped entirely: a per-query constant logit shift
cancels in softmax). Causality:
  - 8 triangular chunk-slots use bf16 masks multiplied post-exp (DVE)
  - core-dependent all-zero/all-one chunk-slots use pre-flagged V
    copies (vt2/vt3), so no mask work at all
K-projection is interleaved per-head-pair with attention so the PE
stays busy during the exp-bound softmax stream.
"""
import numpy as np
import ml_dtypes

B, T, C = 4, 2048, 1024
H, D, FF = 16, 64, 4096
NC = 8
NKC = C // 128     # 8 feature chunks
NFFC = FF // 128   # 32
OWN = 1024
EPS = 1e-5

_STATE = {}


def _build_program():
    import concourse.bacc as bacc
    import concourse.mybir as mybir
    from concourse.tile import TileContext

    F32R = mybir.dt.float32r
    F32 = mybir.dt.float32
    BF16 = mybir.dt.bfloat16
    AF = mybir.ActivationFunctionType
    OP = mybir.AluOpType

    nc = bacc.Bacc("TRN2", target_bir_lowering=False, debug=False,
                   num_devices=NC)

    xbf_d = nc.dram_tensor("xbf", [128, NKC, T], BF16, kind="ExternalInput")
    xq_d = nc.dram_tensor("xq", [128, NKC, OWN], F32, kind="ExternalInput")
    wq_d = nc.dram_tensor("wq", [8, 128, NKC, 128], BF16, kind="ExternalInput")
    wk_d = nc.dram_tensor("wk", [8, 128, NKC, 128], BF16, kind="ExternalInput")
    wv_d = nc.dram_tensor("wv", [128, NKC, C], BF16, kind="ExternalInput")
    wp_d = nc.dram_tensor("wp", [8, 128, NKC, 128], BF16, kind="ExternalInput")
    wf1_d = nc.dram_tensor("wf1", [NFFC, 128, NKC, 128], BF16,
                           kind="ExternalInput")
    wf2_d = nc.dram_tensor("wf2", [NKC, 128, NFFC, 128], BF16,
                           kind="ExternalInput")
    qb_d = nc.dram_tensor("qb", [128, 8], F32, kind="ExternalInput")
    bp_d = nc.dram_tensor("bp", [128, NKC], F32, kind="ExternalInput")
    bf1_d = nc.dram_tensor("bf1", [128, NFFC], F32, kind="ExternalInput")
    bf2_d = nc.dram_tensor("bf2", [128, NKC], F32, kind="ExternalInput")
    # 4 triangular mask slots (same pattern set on every core)
    mask_d = nc.dram_tensor("masks", [128, 4, 512], BF16,
                            kind="ExternalInput")
    fa_d = nc.dram_tensor("fa", [128, 64], F32, kind="ExternalInput")
    fb_d = nc.dram_tensor("fb", [128, 64], F32, kind="ExternalInput")
    out_d = nc.dram_tensor("out", [128, NKC, OWN], F32, kind="ExternalOutput")

    def mm(ps, lhsT, rhs, start, stop):
        nc.tensor.matmul(ps, lhsT, rhs, start=start, stop=stop)

    with TileContext(nc, pool_alloc_mode="queue") as tc:
        consts_cm = tc.tile_pool(name="consts", bufs=1)
        consts = consts_cm.__enter__()

        ones128 = consts.tile([128, 1], F32R)
        nc.vector.memset(ones128.bitcast(F32), 1.0)
        ones128b = consts.tile([128, 1], BF16)
        nc.vector.memset(ones128b, 1.0)
        onesrow = consts.tile([1, 128], F32R)
        nc.vector.memset(onesrow.bitcast(F32), 1.0)
        negrow = consts.tile([1, 128], F32R)
        nc.vector.memset(negrow.bitcast(F32), -1.0)
        eps_t = consts.tile([1, 1], F32)
        nc.vector.memset(eps_t, EPS)
        qb_t = consts.tile([128, 8], F32)
        bp_t = consts.tile([128, NKC], F32)
        bf1_t = consts.tile([128, NFFC], F32)
        bf2_t = consts.tile([128, NKC], F32)
        fa_t = consts.tile([128, 64], F32)
        fb_t = consts.tile([128, 64], F32)

        # x2t/yt outlive the attention pools -> their pools open first
        x2_cm = tc.tile_pool(name="x2p", bufs=1)
        x2p = x2_cm.__enter__()
        x2t = x2p.tile([128, NKC, OWN], F32R)
        yt_cm = tc.tile_pool(name="ytp", bufs=1)
        ytp = yt_cm.__enter__()
        yt = ytp.tile([128, NKC, OWN], BF16)

        # attention-lifetime activation tiles (closed after proj)
        pers_cm = tc.tile_pool(name="pers", bufs=1)
        pers = pers_cm.__enter__()
        lnx = [pers.tile([128, NKC, 512], BF16, tag=f"lnx{tb}",
                         name=f"lnx{tb}") for tb in range(4)]
        vt = [pers.tile([128, 16, 8, 65], BF16, tag=f"vt{g}",
                        name=f"vt{g}") for g in range(2)]
        vt2 = pers.tile([128, 4, 16, 65], BF16, tag="vt2")
        vt3 = pers.tile([128, 4, 16, 65], BF16, tag="vt3")
        qts = [pers.tile([128, OWN], BF16, tag=f"q{hp}", name=f"q{hp}")
               for hp in range(8)]

        # ------- Phases A-C interleaved: LN1 + V + Q per token-block -------
        xtb_cm = tc.tile_pool(name="xtbp", bufs=2)
        xtbp = xtb_cm.__enter__()
        lw_cm = tc.tile_pool(name="lnw", bufs=2)
        lnw = lw_cm.__enter__()
        lst_cm = tc.tile_pool(name="lnst", bufs=1)
        lst = lst_cm.__enter__()
        wvp_cm = tc.tile_pool(name="wvp", bufs=1)
        wvp = wvp_cm.__enter__()
        wv_t = wvp.tile([128, NKC, C], BF16)
        qw_cm = tc.tile_pool(name="qw", bufs=3)
        qw = qw_cm.__enter__()
        lps_cm = tc.tile_pool(name="lnps", bufs=1, space="PSUM")
        lps = lps_cm.__enter__()
        pps_cm = tc.tile_pool(name="projps", bufs=2, space="PSUM")
        pjps = pps_cm.__enter__()

        nc.vector.memset(vt[0][:, :, :, 64:65], 1.0)
        nc.vector.memset(vt[1][:, :, :, 64:65], 1.0)
        def ln1_tb(tb):
            sl = slice(tb * 512, (tb + 1) * 512)
            xtb = xtbp.tile([128, NKC, 512], BF16, tag="xtb")
            nc.sync.dma_start(out=xtb, in_=xbf_d[:, :, sl])
            ps_s = lps.tile([1, 512], F32, tag="ps_s")
            for k in range(NKC):
                mm(ps_s, ones128b, xtb[:, k, :], k == 0, k == NKC - 1)
            sqs = []
            for k in range(NKC):
                sq = lnw.tile([128, 512], BF16, tag=f"sq{k % 2}")
                nc.scalar.activation(out=sq, in_=xtb[:, k, :], func=AF.Square)
                sqs.append(sq)
            ps_q = lps.tile([1, 512], F32, tag="ps_q")
            for k in range(NKC):
                mm(ps_q, ones128b, sqs[k], k == 0, k == NKC - 1)
            mu = lst.tile([1, 512], F32R, tag="mu")
            nc.vector.tensor_scalar_mul(out=mu, in0=ps_s, scalar1=1.0 / C)
            msq = lst.tile([1, 512], F32, tag="msq")
            nc.vector.tensor_scalar_mul(out=msq, in0=ps_q, scalar1=1.0 / C)
            var = lst.tile([1, 512], F32, tag="var")
            nc.vector.tensor_mul(out=var, in0=mu.bitcast(F32),
                                 in1=mu.bitcast(F32))
            nc.vector.tensor_sub(out=var, in0=msq, in1=var)
            nc.scalar.activation(out=msq, in_=var, func=AF.Sqrt,
                                 bias=eps_t, scale=1.0)
            rstd = lst.tile([1, 512], F32R, tag="rstd")
            with nc.allow_low_precision(reason="f32r rstd"):
                nc.vector.reciprocal(out=rstd, in_=msq)
            pmr = lst.tile([1, 512], F32R, tag="pmr")
            with nc.allow_low_precision(reason="f32r pmr"):
                nc.vector.tensor_mul(out=pmr, in0=mu.bitcast(F32),
                                     in1=rstd.bitcast(F32))
            ps_r = lps.tile([128, 512], F32, tag="ps_r")
            mm(ps_r, onesrow, rstd, True, True)
            ps_n = lps.tile([128, 512], F32, tag="ps_n")
            mm(ps_n, negrow, pmr, True, True)
            rb_s = lnw.tile([128, 512], BF16, tag="rb_s")
            nc.scalar.activation(out=rb_s, in_=ps_r, func=AF.Copy)
            nb_s = lnw.tile([128, 512], BF16, tag="nb_s")
            nc.scalar.activation(out=nb_s, in_=ps_n, func=AF.Copy)
            for k in range(NKC):
                nc.vector.tensor_mul(out=lnx[tb][:, k, :], in0=xtb[:, k, :],
                                     in1=rb_s)
                nc.vector.tensor_add(out=lnx[tb][:, k, :],
                                     in0=lnx[tb][:, k, :], in1=nb_s)

        def v_chunk(cch):
            tbi, coff = cch // 4, (cch % 4) * 128
            ps_v = pjps.tile([128, C], F32, tag="pv")
            for h2 in range(2):
                vsl = slice(h2 * 512, (h2 + 1) * 512)
                for k in range(NKC):
                    mm(ps_v[:, vsl], lnx[tbi][:, k, coff:coff + 128],
                       wv_t[:, k, vsl], k == 0, k == NKC - 1)
            for g in range(2):
                nc.scalar.activation(
                    out=vt[g][:, cch, :, 0:64],
                    in_=ps_v[:, g * 512:(g + 1) * 512]
                    .rearrange("p (h d) -> p h d", h=8), func=AF.Copy)
            if 8 <= cch < 12:
                nc.scalar.activation(
                    out=vt2[:, cch - 8, :, 0:64],
                    in_=ps_v.rearrange("p (h d) -> p h d", h=16),
                    func=AF.Copy, scale=fa_t[:, 0:1])
            if 12 <= cch:
                nc.scalar.activation(
                    out=vt3[:, cch - 12, :, 0:64],
                    in_=ps_v.rearrange("p (h d) -> p h d", h=16),
                    func=AF.Copy, scale=fb_t[:, 0:1])

        def q_hp(hp):
            wt = qw.tile([128, NKC, 128], BF16, tag="w")
            nc.sync.dma_start(out=wt, in_=wq_d[hp])
            ps = pjps.tile([128, OWN], F32, tag="pv")
            for tb in range(2):
                sl = slice(tb * 512, (tb + 1) * 512)
                for k in range(NKC):
                    mm(ps[:, sl], wt[:, k, :], lnx[tb][:, k, :],
                       k == 0, k == NKC - 1)
            nc.vector.tensor_scalar_add(out=qts[hp], in0=ps,
                                        scalar1=qb_t[:, hp:hp + 1])

        for tb in range(4):
            ln1_tb(tb)
            if tb == 0:
                nc.sync.dma_start(out=wv_t, in_=wv_d[:, :, :])
                nc.sync.dma_start(out=fa_t, in_=fa_d[:, :])
                nc.sync.dma_start(out=fb_t, in_=fb_d[:, :])
                nc.sync.dma_start(out=qb_t, in_=qb_d[:, :])
                nc.sync.dma_start(out=bp_t, in_=bp_d[:, :])
                nc.sync.dma_start(out=bf1_t, in_=bf1_d[:, :])
                nc.sync.dma_start(out=bf2_t, in_=bf2_d[:, :])
                nc.gpsimd.tensor_copy(
                    out=vt2[:, :, :, 64:65]
                    .rearrange("p a h o -> p (a h o)"), in_=fa_t)
                nc.gpsimd.tensor_copy(
                    out=vt3[:, :, :, 64:65]
                    .rearrange("p a h o -> p (a h o)"), in_=fb_t)
            if tb >= 1:
                for cch in range(4 * (tb - 1), 4 * tb):
                    v_chunk(cch)
            if tb == 2:
                for hp in range(4):
                    q_hp(hp)
            if tb == 3:
                for hp in range(4, 8):
                    q_hp(hp)
        for cch in range(12, 16):
            v_chunk(cch)

        pps_cm.__exit__(None, None, None)
        lps_cm.__exit__(None, None, None)
        qw_cm.__exit__(None, None, None)
        wvp_cm.__exit__(None, None, None)
        lst_cm.__exit__(None, None, None)
        lw_cm.__exit__(None, None, None)
        xtb_cm.__exit__(None, None, None)

        # ---------- Phase D: K projection + attention, per head-pair ----------
        # qb0 slots: chunks 0..3 (tri, mask slot=ci), 8..11 (flagged vt2)
        # qb1 slots: chunks 0..3, 8..11 (plain), 4..7 (tri, slot=ci-4),
        #            12..15 (flagged vt3)
        SLOTS = {
            0: [(ci, "tri", ci) for ci in range(4)]
               + [(ci, "v2", None) for ci in range(8, 12)],
            1: [(ci, "one", None) for ci in range(4)]
               + [(ci, "tri", ci - 4) for ci in range(4, 8)]
               + [(ci, "one", None) for ci in range(8, 12)]
               + [(ci, "v3", None) for ci in range(12, 16)],
        }

        kw_cm = tc.tile_pool(name="kw", bufs=3)
        kw = kw_cm.__enter__()
        mask_t = kw.tile([128, 4, 512], BF16, tag="mask", bufs=1)
        nc.sync.dma_start(out=mask_t, in_=mask_d[:, :, :])
        kt_cm = tc.tile_pool(name="ktp", bufs=3)
        ktp = kt_cm.__enter__()
        att_cm = tc.tile_pool(name="attw", bufs=4)
        attw = att_cm.__enter__()
        sm_cm = tc.tile_pool(name="smw", bufs=2)
        smw = sm_cm.__enter__()
        kps_cm = tc.tile_pool(name="kps", bufs=2, space="PSUM")
        kps = kps_cm.__enter__()
        sps_cm = tc.tile_pool(name="sps", bufs=2, space="PSUM")
        sps = sps_cm.__enter__()
        yps_cm = tc.tile_pool(name="yps", bufs=1, space="PSUM")
        yps = yps_cm.__enter__()

        for hp in range(8):
            ha, hb = 2 * hp, 2 * hp + 1
            g = hp // 4
            wt = kw.tile([128, NKC, 128], BF16, tag="w")
            nc.sync.dma_start(out=wt, in_=wk_d[hp])
            kt_t = ktp.tile([128, T], BF16, tag="kt")
            for tb in range(4):
                sl = slice(tb * 512, (tb + 1) * 512)
                ps_k = kps.tile([128, 512], F32, tag="mm")
                for k in range(NKC):
                    mm(ps_k, wt[:, k, :], lnx[tb][:, k, :],
                       k == 0, k == NKC - 1)
                nc.vector.tensor_copy(out=kt_t[:, sl], in_=ps_k)

            for qb in (1, 0):
                slots = SLOTS[qb]
                nsl = len(slots)
                qsl = slice(qb * 512, (qb + 1) * 512)
                ps_y = yps.tile([65, 2, 512], F32, tag="y")
                for idx, (ci, kind, mslot) in enumerate(slots):
                    csl = slice(ci * 128, (ci + 1) * 128)
                    ps_s = sps.tile([128, 2, 512], F32, tag="s")
                    mm(ps_s[:, 0, :], kt_t[0:64, csl], qts[hp][0:64, qsl],
                       True, True)
                    mm(ps_s[:, 1, :], kt_t[64:128, csl], qts[hp][64:128, qsl],
                       True, True)
                    ptm = attw.tile([128, 2, 512], BF16, tag="ptm")
                    if kind == "tri":
                        pt = attw.tile([128, 2, 512], BF16, tag="pt")
                        nc.scalar.activation(out=pt, in_=ps_s, func=AF.Exp)
                        nc.vector.tensor_mul(
                            out=ptm, in0=pt,
                            in1=mask_t[:, mslot:mslot + 1, :]
                            .broadcast_to([128, 2, 512]))
                    else:
                        nc.scalar.activation(out=ptm, in_=ps_s, func=AF.Exp)
                    if kind == "v2":
                        va, vb = vt2[:, ci - 8, ha, :], vt2[:, ci - 8, hb, :]
                    elif kind == "v3":
                        va, vb = vt3[:, ci - 12, ha, :], vt3[:, ci - 12, hb, :]
                    else:
                        va = vt[g][:, ci, ha - g * 8, :]
                        vb = vt[g][:, ci, hb - g * 8, :]
                    mm(ps_y[:, 0, :], va, ptm[:, 0, :], idx == 0,
                       idx == nsl - 1)
                    mm(ps_y[:, 1, :], vb, ptm[:, 1, :], idx == 0,
                       idx == nsl - 1)
                yraw = smw.tile([65, 2, 512], F32, tag="yraw")
                nc.vector.tensor_copy(out=yraw, in_=ps_y)
                for hh, h in ((0, ha), (1, hb)):
                    rd = smw.tile([1, 512], F32R, tag="rd")
                    with nc.allow_low_precision(reason="softmax denom"):
                        nc.vector.reciprocal(out=rd, in_=yraw[64:65, hh, :])
                    rb = smw.tile([64, 512], F32, tag="rb")
                    nc.gpsimd.partition_broadcast(rb, rd.bitcast(F32))
                    nc.vector.tensor_mul(
                        out=yt[64 * hh:64 * hh + 64, hp, qsl],
                        in0=yraw[0:64, hh, :], in1=rb)

        yps_cm.__exit__(None, None, None)
        sps_cm.__exit__(None, None, None)
        kps_cm.__exit__(None, None, None)
        sm_cm.__exit__(None, None, None)
        att_cm.__exit__(None, None, None)
        kt_cm.__exit__(None, None, None)
        kw_cm.__exit__(None, None, None)
        pers_cm.__exit__(None, None, None)

        # ------- Phases E/F/G interleaved over 512-token halves -------
        l2x_cm = tc.tile_pool(name="l2x", bufs=1)
        l2xp = l2x_cm.__enter__()
        ln2x = l2xp.tile([128, NKC, OWN], BF16)
        m1_cm = tc.tile_pool(name="m1p", bufs=1)
        m1p = m1_cm.__enter__()
        m1t = m1p.tile([128, NFFC, OWN], BF16)
        mw_cm = tc.tile_pool(name="mw", bufs=3)
        mw = mw_cm.__enter__()
        mo_cm = tc.tile_pool(name="mo", bufs=2)
        mo = mo_cm.__enter__()
        mps_cm = tc.tile_pool(name="mps", bufs=2, space="PSUM")
        mps = mps_cm.__enter__()
        pw_cm = tc.tile_pool(name="pw", bufs=3)
        pw = pw_cm.__enter__()
        l2w_cm = tc.tile_pool(name="l2w", bufs=2)
        l2w = l2w_cm.__enter__()
        l2ps_cm = tc.tile_pool(name="l2ps", bufs=1, space="PSUM")
        l2ps = l2ps_cm.__enter__()

        def proj_half(h2):
            tsl = slice(h2 * 512, (h2 + 1) * 512)
            for oc in range(8):
                wt = pw.tile([128, NKC, 128], BF16, tag="w")
                nc.sync.dma_start(out=wt, in_=wp_d[oc])
                xqo = pw.tile([128, 512], F32, tag="xq")
                nc.sync.dma_start(out=xqo, in_=xq_d[:, oc, tsl])
                ps = mps.tile([128, 512], F32, tag="mm1")
                for k in range(NKC):
                    mm(ps, wt[:, k, :], yt[:, k, tsl], k == 0, k == NKC - 1)
                with nc.allow_low_precision(reason="f32r x2"):
                    nc.vector.scalar_tensor_tensor(
                        out=x2t[:, oc, tsl], in0=ps,
                        scalar=bp_t[:, oc:oc + 1],
                        in1=xqo, op0=OP.add, op1=OP.add)

        def ln2_stats(h2):
            tsl = slice(h2 * 512, (h2 + 1) * 512)
            ps_s2 = l2ps.tile([1, 512], F32, tag="ps_s")
            for k in range(NKC):
                mm(ps_s2, ones128, x2t[:, k, tsl], k == 0, k == NKC - 1)
            sq2s = []
            for k in range(NKC):
                sq = l2w.tile([128, 512], BF16, tag=f"sq{k % 2}")
                nc.scalar.activation(out=sq, in_=x2t[:, k, tsl].bitcast(F32),
                                     func=AF.Square)
                sq2s.append(sq)
            ps_q2 = l2ps.tile([1, 512], F32, tag="ps_q")
            for k in range(NKC):
                mm(ps_q2, ones128b, sq2s[k], k == 0, k == NKC - 1)
            mu2_ = l2w.tile([1, 512], F32R, tag="mu")
            nc.vector.tensor_scalar_mul(out=mu2_, in0=ps_s2, scalar1=1.0 / C)
            msq2 = l2w.tile([1, 512], F32, tag="msq")
            nc.vector.tensor_scalar_mul(out=msq2, in0=ps_q2, scalar1=1.0 / C)
            var2 = l2w.tile([1, 512], F32, tag="var")
            nc.vector.tensor_mul(out=var2, in0=mu2_.bitcast(F32),
                                 in1=mu2_.bitcast(F32))
            nc.vector.tensor_sub(out=var2, in0=msq2, in1=var2)
            nc.scalar.activation(out=msq2, in_=var2, func=AF.Sqrt,
                                 bias=eps_t, scale=1.0)
            rstd2 = l2w.tile([1, 512], F32R, tag="rstd")
            with nc.allow_low_precision(reason="f32r rstd"):
                nc.vector.reciprocal(out=rstd2, in_=msq2)
            pmr2 = l2w.tile([1, 512], F32R, tag="pmr")
            with nc.allow_low_precision(reason="f32r pmr"):
                nc.vector.tensor_mul(out=pmr2, in0=mu2_.bitcast(F32),
                                     in1=rstd2.bitcast(F32))
            return rstd2, pmr2

        def ln2_apply(h2, rstd2, pmr2):
            tsl = slice(h2 * 512, (h2 + 1) * 512)
            ps_r2 = l2ps.tile([128, 512], F32, tag="ps_r")
            mm(ps_r2, onesrow, rstd2, True, True)
            ps_n2 = l2ps.tile([128, 512], F32, tag="ps_n")
            mm(ps_n2, negrow, pmr2, True, True)
            nb2 = l2w.tile([128, 512], BF16, tag="nb2")
            nc.scalar.activation(out=nb2, in_=ps_n2, func=AF.Copy)
            for k in range(NKC):
                nc.vector.tensor_mul(out=ln2x[:, k, tsl],
                                     in0=x2t[:, k, tsl].bitcast(F32),
                                     in1=ps_r2)
                nc.vector.tensor_add(out=ln2x[:, k, tsl],
                                     in0=ln2x[:, k, tsl], in1=nb2)

        def fc1_half(h2):
            tsl = slice(h2 * 512, (h2 + 1) * 512)
            for ffc in range(NFFC):
                wt = mw.tile([128, NKC, 128], BF16, tag="w1")
                nc.sync.dma_start(out=wt, in_=wf1_d[ffc])
                ps = mps.tile([128, 512], F32, tag="mm1")
                for k in range(NKC):
                    mm(ps, wt[:, k, :], ln2x[:, k, tsl], k == 0, k == NKC - 1)
                nc.scalar.activation(out=m1t[:, ffc, tsl], in_=ps,
                                     func=AF.Relu,
                                     bias=bf1_t[:, ffc:ffc + 1], scale=1.0)

        proj_half(0)
        st0 = ln2_stats(0)
        ln2_apply(0, *st0)
        proj_half(1)
        st1 = ln2_stats(1)
        fc1_half(0)
        ln2_apply(1, *st1)
        fc1_half(1)

        for oc in range(NKC):
            wt2 = mw.tile([128, NFFC, 128], BF16, tag="w2")
            nc.sync.dma_start(out=wt2, in_=wf2_d[oc])
            ot = mo.tile([128, OWN], F32, tag="ot")
            for h2 in range(2):
                tsl = slice(h2 * 512, (h2 + 1) * 512)
                ps = mps.tile([128, 512], F32, tag="mm2")
                for k in range(NFFC):
                    mm(ps, wt2[:, k, :], m1t[:, k, tsl], k == 0, k == NFFC - 1)
                nc.vector.scalar_tensor_tensor(
                    out=ot[:, tsl], in0=ps, scalar=bf2_t[:, oc:oc + 1],
                    in1=x2t[:, oc, tsl].bitcast(F32), op0=OP.add, op1=OP.add)
            nc.sync.dma_start(out=out_d[:, oc, :], in_=ot)

        l2ps_cm.__exit__(None, None, None)
        l2w_cm.__exit__(None, None, None)
        pw_cm.__exit__(None, None, None)
        mps_cm.__exit__(None, None, None)
        mo_cm.__exit__(None, None, None)
        mw_cm.__exit__(None, None, None)
        m1_cm.__exit__(None, None, None)
        l2x_cm.__exit__(None, None, None)
        yt_cm.__exit__(None, None, None)
        x2_cm.__exit__(None, None, None)
        consts_cm.__exit__(None, None, None)

    nc.compile()
    return nc


def _perm(r):
    if r == 0:
        return np.concatenate([np.arange(0, 512), np.arange(1536, 2048),
                               np.arange(512, 1536)])
    return np.concatenate([np.arange(512, 1536), np.arange(0, 512),
                           np.arange(1536, 2048)])


def _prep_in_maps(x, W_attn, W_proj, b_proj, W_fc1, b_fc1, W_fc2, b_fc2,
                  ln1_g, ln1_b, ln2_g, ln2_b):
    f32 = np.float32
    bf16 = ml_dtypes.bfloat16
    x = np.asarray(x, f32)
    W_attn = np.asarray(W_attn, f32)
    Wq, Wk, Wv = W_attn[:, 0:C], W_attn[:, C:2 * C], W_attn[:, 2 * C:3 * C]
    W_proj = np.asarray(W_proj, f32)
    W_fc1 = np.asarray(W_fc1, f32)
    W_fc2 = np.asarray(W_fc2, f32)
    g1 = np.asarray(ln1_g, f32)
    b1 = np.asarray(ln1_b, f32)
    g2 = np.asarray(ln2_g, f32)
    b2 = np.asarray(ln2_b, f32)

    s = 1.0 / np.sqrt(D)
    Wq_f = (g1[:, None] * Wq) * s
    Wk_f = g1[:, None] * Wk
    Wv_f = g1[:, None] * Wv
    Wf1_f = g2[:, None] * W_fc1
    qbias = (b1 @ Wq) * s                      # [C]
    vbias = b1 @ Wv                            # [C]
    bp_f = np.asarray(b_proj, f32) + vbias @ W_proj
    bf1_f = np.asarray(b_fc1, f32) + b2 @ W_fc1
    bf2_f = np.asarray(b_fc2, f32)

    def lhs_tiles(W, nout):
        nin = W.shape[0] // 128
        return np.ascontiguousarray(
            W.reshape(nin, 128, nout, 128).transpose(2, 1, 0, 3)
        ).astype(bf16)

    def vec(v, nk):
        return np.ascontiguousarray(np.asarray(v, f32).reshape(nk, 128).T)

    kvp = np.arange(128)
    qi = np.arange(512)
    masks = np.zeros((128, 4, 512), np.float32)
    for j in range(4):
        masks[:, j, :] = (128 * j + kvp[:, None]) <= qi[None, :]

    shared = {
        "wq": lhs_tiles(Wq_f, 8), "wk": lhs_tiles(Wk_f, 8),
        "wv": np.ascontiguousarray(
            Wv_f.reshape(NKC, 128, C).transpose(1, 0, 2)).astype(bf16),
        "wp": lhs_tiles(W_proj, 8),
        "wf1": lhs_tiles(Wf1_f, NFFC), "wf2": lhs_tiles(W_fc2, NKC),
        "qb": vec(qbias, 8), "bp": vec(bp_f, NKC),
        "bf1": vec(bf1_f, NFFC), "bf2": vec(bf2_f, NKC),
        "masks": masks.astype(bf16),
    }

    in_maps = []
    for c in range(NC):
        b, r = c // 2, c % 2
        perm = _perm(r)
        xs = x[b][perm]                       # [T, C] permuted
        xt = np.ascontiguousarray(
            xs.T.reshape(NKC, 128, T).transpose(1, 0, 2))
        fa = np.full((128, 64), float(r == 1), np.float32)
        fb = np.full((128, 64), float(r == 0), np.float32)
        d = {"xbf": xt.astype(bf16), "xq": np.ascontiguousarray(xt[:, :, 0:OWN]),
             "fa": fa, "fb": fb}
        d.update(shared)
        in_maps.append(d)
    return in_maps


class _SpmdRunner:
    def __init__(self, nc, n_cores=NC):
        import jax
        from jax.sharding import Mesh, PartitionSpec
        from jax.experimental.shard_map import shard_map
        import concourse.mybir as mybir
        from concourse import bass2jax
        bass2jax.install_neuronx_cc_hook()
        self.jax = jax
        self.n_cores = n_cores
        partition_name = (
            nc.partition_id_tensor.name if nc.partition_id_tensor else None)
        in_names, out_names, out_avals = [], [], []
        for alloc in nc.m.functions[0].allocations:
            if not isinstance(alloc, mybir.MemoryLocationSet):
                continue
            name = alloc.memorylocations[0].name
            if alloc.kind == "ExternalInput":
                if name != partition_name:
                    in_names.append(name)
            elif alloc.kind == "ExternalOutput":
                out_names.append(name)
                out_avals.append(jax.core.ShapedArray(
                    tuple(alloc.tensor_shape), mybir.dt.np(alloc.dtype)))
        self.in_names = in_names
        self.out_names = out_names
        self.out_avals = out_avals
        all_in = in_names + out_names
        if partition_name is not None:
            all_in.append(partition_name)

        def _body(*args):
            operands = list(args)
            if partition_name is not None:
                operands.append(bass2jax.partition_id_tensor())
            outs = bass2jax._bass_exec_p.bind(
                *operands, out_avals=tuple(out_avals),
                in_names=tuple(all_in), out_names=tuple(out_names),
                lowering_input_output_aliases=(),
                sim_require_finite=True, sim_require_nnan=True, nc=nc)
            return tuple(outs)

        devices = jax.devices()[:n_cores]
        self.mesh = Mesh(np.asarray(devices), ("core",))
        n_io = len(in_names) + len(out_names)
        self.fn = jax.jit(
            shard_map(_body, mesh=self.mesh,
                      in_specs=(PartitionSpec("core"),) * n_io,
                      out_specs=(PartitionSpec("core"),) * len(out_names),
                      check_rep=False),
            keep_unused=True)
        self._dev_in = None

    def put_inputs(self, in_maps):
        from jax.sharding import NamedSharding, PartitionSpec
        jax = self.jax
        sh = NamedSharding(self.mesh, PartitionSpec("core"))
        concat = []
        for name in self.in_names:
            arrs = [np.asarray(in_maps[c][name]) for c in range(self.n_cores)]
            concat.append(jax.device_put(np.concatenate(arrs, axis=0), sh))
        for av in self.out_avals:
            z = np.zeros((self.n_cores * av.shape[0], *av.shape[1:]), av.dtype)
            concat.append(jax.device_put(z, sh))
        self._dev_in = concat

    def run(self):
        jax = self.jax
        outs = self.fn(*self._dev_in)
        jax.block_until_ready(outs)
        results = []
        for c in range(self.n_cores):
            d = {}
            for i, name in enumerate(self.out_names):
                av = self.out_avals[i]
                d[name] = np.asarray(outs[i]).reshape(
                    self.n_cores, *av.shape)[c]
            results.append(d)
        return results

    def time_exec(self, warmup=3, m1=4, m2=12, reps=3, trials=6):
        """Estimate per-call device time by differencing burst timings,
        which cancels the constant dispatch/RTT overhead of the axon
        tunnel."""
        import time
        jax = self.jax
        for _ in range(warmup):
            jax.block_until_ready(self.fn(*self._dev_in))

        def burst(m):
            t0 = time.perf_counter()
            outs = None
            for _ in range(m):
                outs = self.fn(*self._dev_in)
            jax.block_until_ready(outs)
            return time.perf_counter() - t0

        t1s, t2s = [], []
        for _ in range(trials):
            for _ in range(reps):
                t1s.append(burst(m1))
                t2s.append(burst(m2))
        return (min(t2s) - min(t1s)) / (m2 - m1)


def _get_runner():
    if "runner" not in _STATE:
        nc = _build_program()
        _STATE["runner"] = _SpmdRunner(nc)
    return _STATE["runner"]


def kernel(x, W_attn, W_proj, b_proj, W_fc1, b_fc1, W_fc2, b_fc2,
           ln1_g, ln1_b, ln2_g, ln2_b):
    runner = _get_runner()
    in_maps = _prep_in_maps(x, W_attn, W_proj, b_proj, W_fc1, b_fc1,
                            W_fc2, b_fc2, ln1_g, ln1_b, ln2_g, ln2_b)
    runner.put_inputs(in_maps)
    results = runner.run()
    out = np.empty((B, T, C), np.float32)
    for c in range(NC):
        b, r = c // 2, c % 2
        ot = results[c]["out"]                # [128, NKC, OWN]
        feat = ot.transpose(1, 0, 2).reshape(C, OWN)
        out[b, _perm(r)[0:OWN], :] = feat.T
    return out
